# revision 9
# baseline (speedup 1.0000x reference)
"""Trainium2 Bass kernel for nn_Attentive (additive attention with sentinel gate).

Reference math (per batch b):
  cv = att @ Wv                    [K, A]
  cg = hid @ Wg                    [T, A]
  z[t,k]   = sum_a tanh(cv[k,a] + cg[t,a]) * wh[a]
  alpha    = softmax_K(z)
  c_t      = alpha @ att           [T, H]
  cs       = sen @ Ws + cg         [T, A]
  z_ext[t] = sum_a tanh(cs[t,a]) * wh[a]
  ext softmax over [z, z_ext] -> beta; c_hat = beta*sen + (1-beta)*c_t

Sharding: data-parallel over batch B=32 across 8 cores (4 batches/core),
weights replicated.

Layout strategy (per core, B_loc=4, T=32 -> bt = b*32+t in [0,128)):
  * hid/sen for all 4 local batches live as one [128, 512] tile (partition=bt).
  * A-dim on partitions for the content/tanh stage: cvT[a,k], cgT[a,bt] via PE
    matmuls against PE-transposed attT/hidT/senT (fp32).
  * content tiles (bf16): DVE tensor_scalar adds (cvT_bf + cgT column), grouped
    G t's per tile so the ACT tanh runs with huge free dims (ACT is the
    bottleneck engine: 1 elem/cycle/lane, dtype-independent, ~352cy/inst
    overhead).
  * z rows via M=1 PE matmuls (stationary wh chunk [128,1], moving tanh tile
    [128,196] bf16), accumulated over the 4 a-chunks in PSUM.
  * softmax/sentinel merge on [128, 197] (all 4 batches at once), c_t via
    PE-transposed alpha, c_hat elementwise on DVE.
"""

import os
import numpy as np

import concourse.bass as bass
from concourse import bacc
import concourse.tile as tile
from concourse import mybir
from concourse.bass_utils import run_bass_kernel_spmd
from concourse.masks import make_identity

F32 = mybir.dt.float32
BF16 = mybir.dt.bfloat16
AF = mybir.ActivationFunctionType
OP = mybir.AluOpType

NCORES = 8
B, T, K, H, A = 32, 32, 196, 512, 512
BL = B // NCORES          # local batches per core = 4
BT = BL * T               # 128
KC0 = 128                 # first k chunk
KC1 = K - KC0             # 68
HC = H // 128             # 4 h chunks
AC = A // 128             # 4 a chunks
G = 8                     # t-group size for the tanh stage
NG = T // G               # groups per batch

# z-row placement plan:
#  "A": M=1 matmuls write psum rows at arbitrary base partition bt with
#       explicit tile_position=(0, 32*(bt//32)).  psum z is [128, 197].
#  "D": rows packed per-b at (strip 32j, 256*q f32) in psum, bounced via DVE
#       to sbuf, then SBUF->SBUF DMA partition-scatter into z_all.
Z_PLAN = os.environ.get("Z_PLAN", "D")


def build(z_plan: str = Z_PLAN) -> bass.Bass:
    nc = bacc.Bacc(None)

    att = nc.dram_tensor("att_feats", [BL, K, H], F32, kind="ExternalInput")
    hid = nc.dram_tensor("hiddens", [BL, T, H], F32, kind="ExternalInput")
    sen = nc.dram_tensor("sentinel", [BL, T, H], F32, kind="ExternalInput")
    Wv = nc.dram_tensor("Wv", [H, A], F32, kind="ExternalInput")
    Wg = nc.dram_tensor("Wg", [H, A], F32, kind="ExternalInput")
    Ws = nc.dram_tensor("Ws", [H, A], F32, kind="ExternalInput")
    wh = nc.dram_tensor("wh", [A], F32, kind="ExternalInput")

    c_hat = nc.dram_tensor("c_hat_t", [BL, T, H], F32, kind="ExternalOutput")
    alpha_o = nc.dram_tensor("alpha_t", [BL, T, K], F32, kind="ExternalOutput")
    beta_o = nc.dram_tensor("beta_t", [BL, T, 1], F32, kind="ExternalOutput")

    with tile.TileContext(nc) as tc:
        with (
            tc.tile_pool(name="big", bufs=1) as big,        # persistent tiles
            tc.tile_pool(name="cont", bufs=2) as cont,      # content tiles
            tc.tile_pool(name="tanh", bufs=2) as tanhp,     # tanh tiles
            tc.tile_pool(name="ps", bufs=2, space="PSUM") as ps,
            tc.tile_pool(name="psz", bufs=1, space="PSUM") as psz,
            tc.tile_pool(name="psct", bufs=1, space="PSUM") as psct,
        ):
            # ---------------- loads ----------------
            hid_sb = big.tile([BT, H], F32, tag="hid")
            nc.sync.dma_start(hid_sb[:], hid[:].rearrange("b t h -> (b t) h"))
            sen_sb = big.tile([BT, H], F32, tag="sen")
            nc.sync.dma_start(sen_sb[:], sen[:].rearrange("b t h -> (b t) h"))

            # att k-chunks: [k_partition, b, h]; chunk1 zero-padded to 128 rows
            att0 = big.tile([128, BL, H], F32, tag="att0")
            nc.sync.dma_start(att0[:], att[:, 0:KC0, :].rearrange("b k h -> k b h"))
            att1 = big.tile([128, BL, H], F32, tag="att1")
            nc.vector.memset(att1[:], 0.0)
            nc.sync.dma_start(
                att1[0:KC1, :, :], att[:, KC0:K, :].rearrange("b k h -> k b h")
            )

            w_sb = {}
            for name, t in (("Wv", Wv), ("Wg", Wg), ("Ws", Ws)):
                w_sb[name] = big.tile([128, HC, A], F32, tag=name, name=f"w_{name}")
                nc.sync.dma_start(
                    w_sb[name][:], t[:].rearrange("(hc p) a -> p hc a", p=128)
                )
            wh_sb = big.tile([128, AC], F32, tag="wh")
            nc.sync.dma_start(wh_sb[:], wh[:].rearrange("(c p) -> p c", p=128))
            wh_bf = big.tile([128, AC], BF16, tag="whbf")
            nc.vector.tensor_copy(wh_bf[:], wh_sb[:])

            ident = big.tile([128, 128], F32, tag="ident")
            make_identity(nc, ident[:])

            # ---------------- transposes (PE) ----------------
            # hidT/senT: [h_inner, hc, bt]
            hidT = big.tile([128, HC, BT], F32, tag="hidT")
            senT = big.tile([128, HC, BT], F32, tag="senT")
            for src, dst in ((hid_sb, hidT), (sen_sb, senT)):
                for hc in range(HC):
                    pt = ps.tile([128, 128], F32, tag="ps")
                    nc.tensor.transpose(pt[:], src[:, hc * 128 : (hc + 1) * 128], ident[:])
                    nc.vector.tensor_copy(dst[:, hc, :], pt[:])

            # attT[b]: [h_inner, hc, k] (196 = 128 + 68)
            attT = [big.tile([128, HC, K], F32, tag=f"attT{b}", name=f"attT{b}") for b in range(BL)]
            for b in range(BL):
                for hc in range(HC):
                    pt = ps.tile([128, 128], F32, tag="ps")
                    nc.tensor.transpose(
                        pt[:], att0[:, b, hc * 128 : (hc + 1) * 128], ident[:]
                    )
                    nc.vector.tensor_copy(attT[b][:, hc, 0:KC0], pt[:])
                    pt2 = ps.tile([128, 128], F32, tag="ps")
                    nc.tensor.transpose(
                        pt2[:], att1[:, b, hc * 128 : (hc + 1) * 128], ident[:]
                    )
                    nc.vector.tensor_copy(attT[b][:, hc, KC0:K], pt2[:, 0:KC1])

            # ---------------- H-contraction matmuls ----------------
            # cgT: [a_inner, ac, bt] bf16 (bias columns for the content adds)
            cgT = big.tile([128, AC, BT], F32, tag="cgT")
            for ac in range(AC):
                pt = ps.tile([128, BT], F32, tag="ps")
                for hc in range(HC):
                    nc.tensor.matmul(
                        pt[:],
                        w_sb["Wg"][:, hc, ac * 128 : (ac + 1) * 128],
                        hidT[:, hc, :],
                        start=(hc == 0),
                        stop=(hc == HC - 1),
                    )
                nc.vector.tensor_copy(cgT[:, ac, :], pt[:])

            # cvT[b]: [a_inner, ac, k] bf16
            cvT = [big.tile([128, AC, K], BF16, tag=f"cvT{b}", name=f"cvT{b}") for b in range(BL)]
            for b in range(BL):
                for ac in range(AC):
                    pt = ps.tile([128, K], F32, tag="ps")
                    for hc in range(HC):
                        nc.tensor.matmul(
                            pt[:],
                            w_sb["Wv"][:, hc, ac * 128 : (ac + 1) * 128],
                            attT[b][:, hc, :],
                            start=(hc == 0),
                            stop=(hc == HC - 1),
                        )
                    nc.vector.tensor_copy(cvT[b][:, ac, :], pt[:])

            # content_s^T = Ws^T senT + cgT: [a_inner, ac, bt] -> tanh -> bf16
            tanh_cs = big.tile([128, AC, BT], BF16, tag="tanhcs")
            for ac in range(AC):
                pt = ps.tile([128, BT], F32, tag="ps")
                for hc in range(HC):
                    nc.tensor.matmul(
                        pt[:],
                        w_sb["Wg"][:, hc, ac * 128 : (ac + 1) * 128],
                        hidT[:, hc, :],
                        start=(hc == 0),
                        stop=False,
                    )
                for hc in range(HC):
                    nc.tensor.matmul(
                        pt[:],
                        w_sb["Ws"][:, hc, ac * 128 : (ac + 1) * 128],
                        senT[:, hc, :],
                        start=False,
                        stop=(hc == HC - 1),
                    )
                nc.scalar.activation(tanh_cs[:, ac, :], pt[:], AF.Tanh)

            # z_ext column: [bt, 1] (M=128 matmul, stationary = tanh_cs chunk)
            zext_ps = ps.tile([BT, 1], F32, tag="ps")
            for ac in range(AC):
                nc.tensor.matmul(
                    zext_ps[:],
                    tanh_cs[:, ac, :],
                    wh_bf[:, ac : ac + 1],
                    start=(ac == 0),
                    stop=(ac == AC - 1),
                )
            zext_sb = big.tile([BT, 1], F32, tag="zext")
            nc.vector.tensor_copy(zext_sb[:], zext_ps[:])

            # ---------------- the big tanh + z phase ----------------
            if z_plan == "A":
                z_ps = psz.tile([BT, 197], F32, tag="z")
            z_all = big.tile([BT, 197], F32, tag="zall")

            for b in range(BL):
                if z_plan == "D":
                    zd_ps = psz.tile([128, 2048], F32, tag="z")
                    zd_view = zd_ps[:].rearrange(
                        "(j r) (q m) -> j r q m", j=4, q=8
                    )
                for g in range(NG):
                    # content tile: [a_inner, G, ac, k] bf16
                    ct = cont.tile([128, G, AC, K], BF16, tag="cont")
                    for gg in range(G):
                        t = g * G + gg
                        bt = b * T + t
                        for ac in range(AC):
                            nc.vector.tensor_scalar_add(
                                ct[:, gg, ac, :],
                                cvT[b][:, ac, :],
                                cgT[:, ac, bt : bt + 1],
                            )
                    th = tanhp.tile([128, G, AC, K], BF16, tag="tanh")
                    nc.scalar.activation(th[:], ct[:], AF.Tanh)
                    for gg in range(G):
                        t = g * G + gg
                        bt = b * T + t
                        if z_plan == "A":
                            out_row = z_ps[bt : bt + 1, 0:K]
                            tp = (0, 32 * (bt // 32))
                        else:
                            j, q = t % 4, t // 4
                            out_row = zd_view[j, 0:1, q, 0:K]
                            tp = (0, 32 * j)
                        for ac in range(AC):
                            nc.tensor.matmul(
                                out_row,
                                wh_bf[:, ac : ac + 1],
                                th[:, gg, ac, :],
                                start=(ac == 0),
                                stop=(ac == AC - 1),
                                tile_position=tp,
                            )
                if z_plan == "D":
                    # lane-aligned bounce psum->sbuf (DVE can't cross
                    # partitions), then SBUF->SBUF DMA partition-scatter
                    zd_sb = cont.tile([128, T // 4, K], F32, tag="zdsb")
                    nc.vector.tensor_copy(
                        zd_sb[:],
                        zd_ps[:].rearrange("p (q m) -> p q m", q=T // 4)[:, :, 0:K],
                    )
                    zd_sv = zd_sb[:].rearrange("(j r) q n -> j r q n", j=4)
                    for q in range(T // 4):
                        nc.sync.dma_start(
                            z_all[b * T + 4 * q : b * T + 4 * q + 4, 0:K],
                            zd_sv[:, 0, q, :],
                        )

            # ---------------- softmax + sentinel merge ----------------
            if z_plan == "A":
                nc.vector.tensor_copy(z_all[:, 0:K], z_ps[:, 0:K])
            nc.vector.tensor_copy(z_all[:, K : K + 1], zext_sb[:])

            m_neg = big.tile([BT, 1], F32, tag="mneg")
            nc.vector.tensor_reduce(
                m_neg[:], z_all[:], axis=mybir.AxisListType.X, op=OP.max, negate=True
            )
            E = big.tile([BT, 197], F32, tag="E")
            nc.scalar.activation(E[:], z_all[:], AF.Exp, bias=m_neg[:, 0:1])
            s196 = big.tile([BT, 1], F32, tag="s196")
            nc.vector.tensor_reduce(
                s196[:], E[:, 0:K], axis=mybir.AxisListType.X, op=OP.add
            )
            denom = big.tile([BT, 1], F32, tag="denom")
            nc.vector.tensor_tensor(denom[:], s196[:], E[:, K : K + 1], OP.add)
            r196 = big.tile([BT, 1], F32, tag="r196")
            nc.vector.reciprocal(r196[:], s196[:])
            rden = big.tile([BT, 1], F32, tag="rden")
            nc.vector.reciprocal(rden[:], denom[:])

            alpha_sb = big.tile([BT, K], F32, tag="alpha")
            nc.vector.tensor_scalar_mul(alpha_sb[:], E[:, 0:K], r196[:, 0:1])
            beta_sb = big.tile([BT, 1], F32, tag="beta")
            nc.vector.tensor_tensor(beta_sb[:], E[:, K : K + 1], rden[:], OP.mult)

            nc.sync.dma_start(alpha_o[:].rearrange("b t k -> (b t) k"), alpha_sb[:])
            nc.sync.dma_start(beta_o[:].rearrange("b t o -> (b t) o"), beta_sb[:])

            # ---------------- c_t = alpha @ att ----------------
            alphaT0 = big.tile([128, BT], F32, tag="alphaT0")
            pt = ps.tile([128, BT], F32, tag="ps")
            nc.tensor.transpose(pt[:], alpha_sb[:, 0:KC0], ident[:])
            nc.vector.tensor_copy(alphaT0[:], pt[:])
            alphaT1 = big.tile([128, BT], F32, tag="alphaT1")
            nc.vector.memset(alphaT1[:], 0.0)
            pt = ps.tile([128, BT], F32, tag="ps")
            nc.tensor.transpose(pt[0:KC1, :], alpha_sb[:, KC0:K], ident[:])
            nc.vector.tensor_copy(alphaT1[0:KC1, :], pt[0:KC1, :])

            ct_ps = psct.tile([BT, H], F32, tag="ct")
            for b in range(BL):
                for kc, aT in ((0, alphaT0), (1, alphaT1)):
                    nc.tensor.matmul(
                        ct_ps[b * T : (b + 1) * T, :],
                        aT[:, b * T : (b + 1) * T],
                        (att0 if kc == 0 else att1)[:, b, :],
                        start=(kc == 0),
                        stop=(kc == 1),
                        tile_position=(0, b * T),
                    )

            # ---------------- c_hat = c_t + beta*(sen - c_t) ----------------
            d_sb = big.tile([BT, H], F32, tag="dsb")
            nc.vector.tensor_tensor(d_sb[:], sen_sb[:], ct_ps[:], OP.subtract)
            ch_sb = big.tile([BT, H], F32, tag="chsb")
            nc.vector.tensor_scalar_mul(ch_sb[:], d_sb[:], beta_sb[:, 0:1])
            nc.vector.tensor_tensor(ch_sb[:], ch_sb[:], ct_ps[:], OP.add)
            nc.sync.dma_start(c_hat[:].rearrange("b t h -> (b t) h"), ch_sb[:])

    nc.finalize()
    return nc


_NC_CACHE: dict[str, bass.Bass] = {}


def _get_nc(z_plan: str) -> bass.Bass:
    if z_plan not in _NC_CACHE:
        _NC_CACHE[z_plan] = build(z_plan)
    return _NC_CACHE[z_plan]


def run(inputs: dict[str, np.ndarray], trace: bool = False, z_plan: str = Z_PLAN):
    nc = _get_nc(z_plan)
    att = np.ascontiguousarray(np.asarray(inputs["att_feats"], dtype=np.float32))
    hid = np.ascontiguousarray(np.asarray(inputs["hiddens"], dtype=np.float32))
    sen = np.ascontiguousarray(np.asarray(inputs["sentinel"], dtype=np.float32))
    Wv = np.ascontiguousarray(np.asarray(inputs["Wv"], dtype=np.float32))
    Wg = np.ascontiguousarray(np.asarray(inputs["Wg"], dtype=np.float32))
    Ws = np.ascontiguousarray(np.asarray(inputs["Ws"], dtype=np.float32))
    wh = np.ascontiguousarray(np.asarray(inputs["wh"], dtype=np.float32))

    in_maps = []
    for c in range(NCORES):
        sl = slice(c * BL, (c + 1) * BL)
        in_maps.append(
            {
                "att_feats": att[sl],
                "hiddens": hid[sl],
                "sentinel": sen[sl],
                "Wv": Wv,
                "Wg": Wg,
                "Ws": Ws,
                "wh": wh,
            }
        )
    res = run_bass_kernel_spmd(nc, in_maps, list(range(NCORES)), trace=trace)
    c_hat = np.concatenate([res.results[c]["c_hat_t"] for c in range(NCORES)], axis=0)
    alpha = np.concatenate([res.results[c]["alpha_t"] for c in range(NCORES)], axis=0)
    beta = np.concatenate([res.results[c]["beta_t"] for c in range(NCORES)], axis=0)
    return (c_hat, alpha, beta), res


def kernel(**inputs) -> tuple[np.ndarray, np.ndarray, np.ndarray]:
    (c_hat, alpha, beta), _ = run(inputs, trace=False)
    return (c_hat, alpha, beta)
